# revision 1
# baseline (speedup 1.0000x reference)
"""EquivariantEvolution kernel for 8 Trainium2 NeuronCores (Bass/Tile).

Math (per sample, reference):
    alpha = Linear2(silu(Linear1(z)))                     # [NG]
    A     = sum_g alpha_g G_g                             # [D, D]
    z_t   = (I + A + A^2/2 + A^3/6 + A^4/24) z            # order-4 Taylor
    h1    = W1 z_t + b1
    out   = W2 (sigmoid(|h1| + eps) * h1) + b2

Device strategy (pure data-parallel over batch, feature-major layout):
  * Host pre-transposes z to [D, B/8] per core; all weight reshuffling is
    done on host so the device only runs matmuls / elementwise ops.
  * Horner:  v <- z + (1/k) A v.  A v is computed as one K=128 contraction:
      y[(g,i), b] = alpha_g[b] * v[i, b]   (DVE tensor_tensor, alpha
      replicated across the 32 i-partitions by construction)
      (A v)[j, b] = sum_{(g,i)} G[g,j,i] y[(g,i),b]   (two K=128 matmuls)
    The matmul lhsT is tiled 4x along M so the output lands pre-replicated
    [(r,j), b] for the next step's elementwise multiply.
  * sigmoid(x) = 0.5 tanh(x/2) + 0.5 keeps everything in the silu ACT
    table set; the lone sqrt is batched across all tiles (2 table switches
    per kernel instead of 2 per tile).
  * k=1 Horner step is fused with the MLP first layer (W1 folded into the
    Taylor weights); the gate multiply is commuted into a doubled final
    matmul: W2(gate*h1) = 0.5 W2 (t*h1) + 0.5 W2 h1.
"""

import os
import sys

import numpy as np

for _p in ("/opt/trn_rl_repo", "/root/.axon_site/_ro/trn_rl_repo"):
    if os.path.isdir(_p) and _p not in sys.path:
        sys.path.insert(0, _p)

import concourse.bass as bass
import concourse.mybir as mybir
import concourse.tile as tile
from concourse.bass_utils import run_bass_kernel_spmd

B, D, H, NG = 65536, 32, 128, 8
NCORES = 8
BC = B // NCORES          # samples per core
BT = 512                  # samples (free-dim columns) per tile
EPS = 1e-6
F32 = mybir.dt.float32
F32R = mybir.dt.float32r
BF16 = mybir.dt.bfloat16
AF = mybir.ActivationFunctionType

# Taylor weights run as bf16 (full-rate PE, overlappable LDWEIGHTS);
# z-path / extractor / MLP stay float32r for precision.
_BF16_PARAMS = ("LT_t4", "LT_b4", "LT_t3", "LT_b3")


def _param_dt(name):
    if name.startswith("B"):
        return F32
    if name in _BF16_PARAMS:
        return BF16
    return F32R


def _r(ap):
    """View an fp32 AP as float32r for single-pass full-rate PE matmuls."""
    return ap.bitcast(F32R)

# weight/bias DRAM parameters: name -> shape
_PARAM_SHAPES = {
    "LT_h": [D, H],          # W_se1^T
    "LT_At": [H, H],         # W_se2[0:4] replicated over i
    "LT_Ab": [H, H],         # W_se2[4:8] replicated over i
    "Bse1": [H, 1],
    "Bse2t": [H, 1],
    "Bse2b": [H, 1],
    "LT_t4": [H, H], "LT_b4": [H, H],
    "LT_t3": [H, H], "LT_b3": [H, H],
    "LT_t2": [H, H], "LT_b2k": [H, H],
    "LT_t1": [H, H], "LT_b1k": [H, H],   # k=1 step fused with W1
    "LT_z": [D, H],          # identity replicated 4x along M
    "LT_W1z": [D, H],        # W1^T
    "B1": [H, 1],
    "LT_W2": [H, D],         # 0.5 * W2^T
    "B2": [D, 1],
}


def _split_multi_waits(nc, max_waits=1):
    """This toolchain's walrus rejects >1 sync-wait on an instruction
    ("Too many sync wait commands"); hoist extra waits onto preceding
    same-engine NOPs (in-order engines make this semantics-preserving)."""
    n_new = 0
    for f in nc.m.functions:
        for bb in f.blocks:
            out = []
            for ins in bb.instructions:
                si = getattr(ins, "sync_info", None)
                if si is not None and si.on_wait and len(si.on_wait) > max_waits:
                    waits = list(si.on_wait)
                    chunks = [waits[i:i + max_waits] for i in range(0, len(waits), max_waits)]
                    for ci, ch in enumerate(chunks[:-1]):
                        nop = mybir.InstNoOp(
                            name=f"{ins.name}-wsplit{ci}",
                            engine=ins.engine,
                            sync_info=mybir.SyncInfo(on_wait=ch, on_update=[]),
                            bass_nofuse=True,
                        )
                        out.append(nop)
                        n_new += 1
                    ins.sync_info = mybir.SyncInfo(on_wait=chunks[-1], on_update=si.on_update)
                out.append(ins)
            bb.instructions[:] = out
    return n_new


def _build_program(bc: int, sim_safe: bool = False, split_waits: bool = True):
    """Trace the per-core Bass program for bc samples.

    sim_safe decomposes Silu into Sigmoid*x (CoreSim has no Silu handler);
    the hardware path uses the native Silu LUT.
    """
    nt = bc // BT
    nc = bass.Bass()

    zT = nc.declare_dram_parameter("zT", [D, bc], F32R, isOutput=False)
    params = {
        name: nc.declare_dram_parameter(name, shape, _param_dt(name), isOutput=False)
        for name, shape in _PARAM_SHAPES.items()
    }
    eall = nc.declare_dram_parameter("E_all", [nt, nt * H], F32R, isOutput=False)
    onsq = nc.declare_dram_parameter("ONES_nsq", [H, nt * nt], F32R, isOutput=False)
    outT = nc.declare_dram_parameter("outT", [D, bc], F32, isOutput=True)

    with tile.TileContext(nc) as tc:
        with (
            tc.tile_pool(name="consts", bufs=1) as consts,
            tc.tile_pool(name="zv4", bufs=4) as zv4_pool,
            tc.tile_pool(name="hs", bufs=3) as hs_pool,
            tc.tile_pool(name="acat", bufs=3) as acat_pool,
            tc.tile_pool(name="ycat", bufs=3) as ycat_pool,
            tc.tile_pool(name="sq", bufs=3) as sq_pool,
            tc.tile_pool(name="h1s", bufs=nt) as h1s_pool,
            tc.tile_pool(name="gate", bufs=1) as gate_pool,
            tc.tile_pool(name="a1g", bufs=2) as a1g_pool,
            tc.tile_pool(name="outs", bufs=3) as outs_pool,
            tc.tile_pool(name="ps", bufs=2, space=bass.MemorySpace.PSUM) as ps_pool,
            tc.tile_pool(name="warm", bufs=1, space=bass.MemorySpace.PSUM) as warm_pool,
            tc.tile_pool(name="pv", bufs=3, space=bass.MemorySpace.PSUM) as pv_pool,
            tc.tile_pool(name="psn", bufs=1, space=bass.MemorySpace.PSUM) as psn_pool,
            tc.tile_pool(name="pso", bufs=1, space=bass.MemorySpace.PSUM) as pso_pool,
        ):
            # ---- load constants into SBUF ----
            ct = {}
            for name, shape in _PARAM_SHAPES.items():
                t = consts.tile(shape, _param_dt(name), name=f"c_{name}")
                nc.sync.dma_start(t[:], params[name][:])
                ct[name] = t
            e_t = consts.tile([nt, nt * H], F32R, name="c_E")
            nc.sync.dma_start(e_t[:], eall[:])
            onsq_t = consts.tile([H, nt * nt], F32R, name="c_onsq")
            nc.sync.dma_start(onsq_t[:], onsq[:])
            zero_b = consts.tile([nt, 1], F32, name="zero_b")
            nc.vector.memset(zero_b[:], 0.0)
            tanh_b = consts.tile([nt, 1], F32, name="tanh_b")
            nc.vector.memset(tanh_b[:], 0.5 * EPS)

            taylor = [
                (ct["LT_t4"], ct["LT_b4"], BF16),
                (ct["LT_t3"], ct["LT_b3"], BF16),
                (ct["LT_t2"], ct["LT_b2k"], F32R),
            ]

            h1s_tiles = []
            nsq_ps = psn_pool.tile([nt, BT], F32, name="nsq_ps", tag="nsq")

            # ---- HAM warm-up: ~17us of dense matmuls pushes the PE clock
            # gate to K=8/8 (2.4 GHz); the main stream's gaps are short
            # enough (<3.4us) to keep it there ----
            wscr = consts.tile([H, BT], BF16, name="wscr")
            nc.vector.memset(wscr[:], 0.0)
            wps = warm_pool.tile([H, BT], F32, name="wps", tag="warm")

            def warm(n, cols=BT):
                for _ in range(n):
                    nc.tensor.matmul(wps[:, 0:cols], ct["LT_t4"][:], wscr[:, 0:cols],
                                     start=True, stop=True)

            warm(64)

            # ================= phase A =================
            # tiles are emitted in pairs, stage-interleaved, so the PE always
            # has the partner tile's matmuls to run while DVE does the y-muls
            for tp in range(0, nt, 2):
                if tp in (6, 12):
                    warm(28)   # re-fire K=8/8 after the ~65us warm budget
                pair = [t for t in (tp, tp + 1) if t < nt]
                zv4s, hss, acats, ycats = {}, {}, {}, {}

                for t in pair:
                    zv4 = zv4_pool.tile([H, BT], F32R, name="zv4")
                    for r in range(4):
                        nc.gpsimd.dma_start(zv4[32 * r:32 * (r + 1), :], zT[:, bass.ts(t, BT)])
                    zv4s[t] = zv4

                for t in pair:
                    hp = ps_pool.tile([H, BT], F32, name="hp", tag="ps")
                    nc.tensor.matmul(hp[:], ct["LT_h"][:], zv4s[t][0:D, :], start=True, stop=True)
                    hs = hs_pool.tile([H, BT], F32R, name="hs")
                    if sim_safe:
                        sg = hs_pool.tile([H, BT], F32, name="sg")
                        nc.scalar.activation(sg[:], hp[:], AF.Sigmoid, bias=ct["Bse1"][:])
                        hx = hs_pool.tile([H, BT], F32, name="hx")
                        nc.scalar.activation(hx[:], hp[:], AF.Identity, bias=ct["Bse1"][:])
                        nc.vector.tensor_tensor(hs[:], sg[:], hx[:], mybir.AluOpType.mult)
                    else:
                        nc.scalar.activation(hs[:], hp[:], AF.Silu, bias=ct["Bse1"][:])
                    hss[t] = hs

                for t in pair:
                    apt = ps_pool.tile([H, BT], F32, name="apt", tag="ps")
                    nc.tensor.matmul(apt[:], ct["LT_At"][:], hss[t][:], start=True, stop=True)
                    apb = ps_pool.tile([H, BT], F32, name="apb", tag="ps")
                    nc.tensor.matmul(apb[:], ct["LT_Ab"][:], hss[t][:], start=True, stop=True)
                    acat = acat_pool.tile([H, 2, BT], F32, name="acat")
                    nc.scalar.activation(acat[:, 0, :], apt[:], AF.Identity, bias=ct["Bse2t"][:])
                    nc.scalar.activation(acat[:, 1, :], apb[:], AF.Identity, bias=ct["Bse2b"][:])
                    acats[t] = acat

                warm(3, 256)
                for t in pair:
                    ycat = ycat_pool.tile([H, 2, BT], BF16, name="ycat0")
                    nc.vector.tensor_tensor(
                        ycat[:], acats[t][:],
                        zv4s[t][:, None, :].broadcast_to([H, 2, BT]),
                        mybir.AluOpType.mult,
                    )
                    ycats[t] = ycat

                for step_i, (lt_top, lt_bot, ydt) in enumerate(taylor):
                    pvs = {}
                    for t in pair:
                        pv = pv_pool.tile([H, BT], F32, name="pv", tag="pv")
                        nc.tensor.matmul(pv[:], lt_top[:], ycats[t][:, 0, :], start=True, stop=False)
                        nc.tensor.matmul(pv[:], lt_bot[:], ycats[t][:, 1, :], start=False, stop=False)
                        nc.tensor.matmul(pv[:], ct["LT_z"][:], zv4s[t][0:D, :], start=False, stop=True)
                        pvs[t] = pv
                    warm(3, 256)
                    nxt = taylor[step_i + 1][2] if step_i + 1 < len(taylor) else F32R
                    for t in pair:
                        ycat = ycat_pool.tile([H, 2, BT], nxt, name="ycat")
                        nc.vector.tensor_tensor(
                            ycat[:], acats[t][:],
                            pvs[t][:, None, :].broadcast_to([H, 2, BT]),
                            mybir.AluOpType.mult,
                        )
                        ycats[t] = ycat

                warm(2, 256)
                h1ps = {}
                for t in pair:
                    h1p = ps_pool.tile([H, BT], F32, name="h1p", tag="ps")
                    nc.tensor.matmul(h1p[:], ct["LT_t1"][:], ycats[t][:, 0, :], start=True, stop=False)
                    nc.tensor.matmul(h1p[:], ct["LT_b1k"][:], ycats[t][:, 1, :], start=False, stop=False)
                    nc.tensor.matmul(h1p[:], ct["LT_W1z"][:], zv4s[t][0:D, :], start=False, stop=True)
                    h1ps[t] = h1p

                warm(2, 256)
                for t in pair:
                    h1s = h1s_pool.tile([H, BT], F32R, name="h1s")
                    nc.scalar.activation(h1s[:], h1ps[t][:], AF.Identity, bias=ct["B1"][:])
                    sq = sq_pool.tile([H, BT], F32R, name="sq")
                    nc.scalar.activation(sq[:], h1ps[t][:], AF.Square, bias=ct["B1"][:])
                    h1s_tiles.append(h1s)
                    nc.tensor.matmul(
                        nsq_ps[:], onsq_t[:, bass.ts(t, nt)], sq[:],
                        start=(t == 0), stop=(t == nt - 1), skip_group_check=True,
                    )

            # ============== gate (batched sqrt + tanh) ==============
            warm(14)
            rt_all = gate_pool.tile([nt, BT], F32, name="rt_all")
            nc.scalar.activation(rt_all[:], nsq_ps[:], AF.Sqrt, bias=zero_b[:])
            t_all = gate_pool.tile([nt, BT], F32R, name="t_all")
            # sigmoid(norm + eps) = 0.5 tanh(0.5 norm + eps/2) + 0.5
            nc.scalar.activation(t_all[:], rt_all[:], AF.Tanh, bias=tanh_b[:], scale=0.5)

            # ================= phase B =================
            for t in range(nt):
                sl = bass.ts(t, BT)
                trp = pso_pool.tile([H, BT], F32, name="trp", tag="pso")
                nc.tensor.matmul(
                    trp[:], e_t[:, bass.ts(t, H)], t_all[:], start=True, stop=True
                )
                a1g = a1g_pool.tile([H, BT], F32R, name="a1g")
                nc.vector.tensor_tensor(
                    a1g[:], h1s_tiles[t][:], trp[:], mybir.AluOpType.mult
                )
                outp = pso_pool.tile([D, BT], F32, name="outp", tag="pso")
                nc.tensor.matmul(outp[:], ct["LT_W2"][:], a1g[:], start=True, stop=False)
                nc.tensor.matmul(outp[:], ct["LT_W2"][:], h1s_tiles[t][:], start=False, stop=True)
                warm(2, 256)
                outs = outs_pool.tile([D, BT], F32, name="outs")
                nc.scalar.activation(outs[:], outp[:], AF.Identity, bias=ct["B2"][:])
                nc.sync.dma_start(outT[:, sl], outs[:])

    if split_waits:
        _split_multi_waits(nc)
    return nc


def _host_params(G, W_se1, b_se1, W_se2, b_se2, W1, b1, W2, b2, nt):
    f = np.float32
    G = np.asarray(G, f)
    Gflat = np.transpose(G, (0, 2, 1)).reshape(NG * D, D)  # [(g,i), j] = G[g,j,i]
    W1G = Gflat @ np.asarray(W1, f).T                      # [(g,i), m]
    p = {
        "LT_h": np.asarray(W_se1, f).T,
        "LT_At": np.repeat(np.asarray(W_se2, f).T[:, 0:4], 32, axis=1),
        "LT_Ab": np.repeat(np.asarray(W_se2, f).T[:, 4:8], 32, axis=1),
        "Bse1": np.asarray(b_se1, f).reshape(H, 1),
        "Bse2t": np.repeat(np.asarray(b_se2, f)[0:4], 32).reshape(H, 1),
        "Bse2b": np.repeat(np.asarray(b_se2, f)[4:8], 32).reshape(H, 1),
        "LT_z": np.tile(np.eye(D, dtype=f), (1, 4)),
        "LT_W1z": np.asarray(W1, f).T,
        "B1": np.asarray(b1, f).reshape(H, 1),
        "LT_W2": 0.5 * np.asarray(W2, f).T,
        "B2": np.asarray(b2, f).reshape(D, 1),
        "LT_t1": np.ascontiguousarray(W1G[:H]),
        "LT_b1k": np.ascontiguousarray(W1G[H:]),
    }
    for k, tname, bname in ((4, "LT_t4", "LT_b4"), (3, "LT_t3", "LT_b3"), (2, "LT_t2", "LT_b2k")):
        scaled = np.tile(Gflat * f(1.0 / k), (1, 4))
        p[tname] = np.ascontiguousarray(scaled[:H])
        p[bname] = np.ascontiguousarray(scaled[H:])
    p["E_all"] = np.ascontiguousarray(np.repeat(np.eye(nt, dtype=f), H, axis=1))
    import ml_dtypes
    for name in _BF16_PARAMS:
        p[name] = p[name].astype(ml_dtypes.bfloat16)
    p["ONES_nsq"] = np.ascontiguousarray(np.tile(np.eye(nt, dtype=f).reshape(1, nt * nt), (H, 1)))
    return p


def _run(z, G, W_se1, b_se1, W_se2, b_se2, W1, b1, W2, b2, trace=False, **trace_kw):
    z = np.asarray(z, np.float32)
    nt = BC // BT
    params = _host_params(G, W_se1, b_se1, W_se2, b_se2, W1, b1, W2, b2, nt)

    # shard: per-core feature-major slices
    zT = np.ascontiguousarray(z.reshape(NCORES, BC, D).transpose(0, 2, 1))

    nc = _build_program(BC)
    in_maps = [{"zT": zT[c], **params} for c in range(NCORES)]
    res = run_bass_kernel_spmd(nc, in_maps, list(range(NCORES)), trace=trace, **trace_kw)

    outT = np.stack([res.results[c]["outT"] for c in range(NCORES)])
    out = outT.transpose(0, 2, 1).reshape(B, D)
    return np.ascontiguousarray(out.astype(np.float32)), res


def kernel(z, G, W_se1, b_se1, W_se2, b_se2, W1, b1, W2, b2):
    out, _ = _run(z, G, W_se1, b_se1, W_se2, b_se2, W1, b1, W2, b2, trace=False)
    return out


if __name__ == "__main__":
    rng = np.random.default_rng(0)
    inputs = {
        "z": rng.standard_normal((B, D), dtype=np.float32),
        "G": (rng.standard_normal((NG, D, D)) * 0.1).astype(np.float32),
        "W_se1": (rng.standard_normal((H, D)) / np.sqrt(D)).astype(np.float32),
        "b_se1": np.zeros(H, np.float32),
        "W_se2": (rng.standard_normal((NG, H)) / np.sqrt(H)).astype(np.float32),
        "b_se2": np.zeros(NG, np.float32),
        "W1": (rng.standard_normal((H, D)) * 0.01).astype(np.float32),
        "b1": np.zeros(H, np.float32),
        "W2": (rng.standard_normal((D, H)) * 0.01).astype(np.float32),
        "b2": np.zeros(D, np.float32),
    }
    out = kernel(**inputs)
    print("kernel output", out.shape, out.dtype, float(np.abs(out).max()))



# revision 9
# speedup vs baseline: 1.7197x; 1.7197x over previous
"""EquivariantEvolution kernel for 8 Trainium2 NeuronCores (Bass/Tile).

Math (per sample, reference):
    alpha = Linear2(silu(Linear1(z)))                     # [NG]
    A     = sum_g alpha_g G_g                             # [D, D]
    z_t   = (I + A + A^2/2 + A^3/6 + A^4/24) z            # order-4 Taylor
    h1    = W1 z_t + b1
    out   = W2 (sigmoid(|h1| + eps) * h1) + b2

Device strategy (pure data-parallel over batch, feature-major layout):
  * Host pre-transposes z to [D, B/8] bf16 per core; weights are reshaped
    on host so the device runs only matmuls / elementwise ops, all in bf16
    (PSUM accumulation stays fp32).
  * Horner:  v <- z + (1/k) A v.  A v is one K=128 contraction:
      y[(g,i), b] = alpha_g[b] * v[i, b]   (DVE tensor_tensor; alpha
      replicated across the 32 i-partitions by construction)
      (A v)[j, b] = sum_{(g,i)} G[g,j,i] y[(g,i),b]   (two K=128 matmuls)
    lhsT is tiled 4x along M so the output lands pre-replicated for the
    next step's elementwise multiply; the z-add is a third K=32 matmul.
  * HAM discipline: a short zero-weight matmul burst at t~0 raises the PE
    clock gate to K=8/8 (~3.4us of activity); the main stream is dense
    bf16 matmuls with no long PE gaps, so the gate stays open.  The only
    structural PE bubble (the sqrt/tanh ACT-table switches before phase B)
    is bridged with a small second burst.
  * k=1 Horner step is fused with the MLP first layer; the gate multiply
    sigmoid(norm+eps) = 0.5 tanh(norm/2 + eps/2) + 0.5 is produced
    broadcast to [H, BT] by a single K=17 matmul (16 tanh rows + a
    constant-ones row carrying the +0.5), so phase B is one gate matmul,
    one DVE multiply and one output matmul per tile, with outputs of 4
    tiles packed into one PSUM bank before a single bias+store pass.
"""

import os
import sys

import numpy as np

for _p in ("/opt/trn_rl_repo", "/root/.axon_site/_ro/trn_rl_repo"):
    if os.path.isdir(_p) and _p not in sys.path:
        sys.path.insert(0, _p)

import concourse.bass as bass
import concourse.mybir as mybir
import concourse.tile as tile
from concourse.bass_utils import run_bass_kernel_spmd

B, D, H, NG = 65536, 32, 128, 8
NCORES = 8
BC = B // NCORES          # samples per core
BT = 512                  # samples (free-dim columns) per tile
EPS = 1e-6
F32 = mybir.dt.float32
BF16 = mybir.dt.bfloat16
AF = mybir.ActivationFunctionType

# weight/bias DRAM parameters: name -> (shape, dtype); nt-dependent ones
# are added in _param_shapes.
def _param_shapes(nt):
    return {
        "LT_h": ([D, H], BF16),          # W_se1^T
        "LT_At": ([H, H], BF16),         # W_se2[0:4] replicated over i
        "LT_Ab": ([H, H], BF16),         # W_se2[4:8] replicated over i
        "Bse1": ([H, 1], F32),
        "Bse2t": ([H, 1], F32),
        "Bse2b": ([H, 1], F32),
        "LT_t4": ([H, H], BF16), "LT_b4": ([H, H], BF16),
        "LT_t3": ([H, H], BF16), "LT_b3": ([H, H], BF16),
        "LT_t2": ([H, H], BF16), "LT_b2k": ([H, H], BF16),
        "LT_t1": ([H, H], BF16), "LT_b1k": ([H, H], BF16),  # k=1 + W1 fused
        "LT_z": ([D, H], BF16),          # identity replicated 4x along M
        "LT_W1z": ([D, H], BF16),        # W1^T
        "B1": ([H, 1], F32),
        "LT_W2": ([H, D], BF16),         # W2^T (0.5 lives in E_sig)
        "B2_2": ([2 * D, 1], F32),       # b2 tiled 2x (2 tiles per bank)
        "E_sig": ([nt + 1, nt * H], BF16),   # sigmoid-broadcast weights
        "ONSQ": ([H, nt * nt], BF16),        # per-tile colsum selectors
    }


def _split_multi_waits(nc, max_waits=1):
    """This toolchain's walrus rejects >1 sync-wait on an instruction
    ("Too many sync wait commands"); hoist extra waits onto preceding
    same-engine NOPs (in-order engines make this semantics-preserving)."""
    n_new = 0
    for f in nc.m.functions:
        for bb in f.blocks:
            out = []
            for ins in bb.instructions:
                si = getattr(ins, "sync_info", None)
                if si is not None and si.on_wait and len(si.on_wait) > max_waits:
                    waits = list(si.on_wait)
                    chunks = [waits[i:i + max_waits] for i in range(0, len(waits), max_waits)]
                    for ci, ch in enumerate(chunks[:-1]):
                        nop = mybir.InstNoOp(
                            name=f"{ins.name}-wsplit{ci}",
                            engine=ins.engine,
                            sync_info=mybir.SyncInfo(on_wait=ch, on_update=[]),
                            bass_nofuse=True,
                        )
                        out.append(nop)
                        n_new += 1
                    ins.sync_info = mybir.SyncInfo(on_wait=chunks[-1], on_update=si.on_update)
                out.append(ins)
            bb.instructions[:] = out
    return n_new


def _build_program(bc: int, sim_safe: bool = False, split_waits: bool = True):
    """Trace the per-core Bass program for bc samples.

    sim_safe decomposes Silu into Sigmoid*x (CoreSim has no Silu handler);
    the hardware path uses the native Silu LUT.
    """
    nt = bc // BT
    ng2 = nt // 2             # output 2-tile groups
    nc = bass.Bass()

    pshapes = _param_shapes(nt)
    zT = nc.declare_dram_parameter("zT", [D, bc], BF16, isOutput=False)
    params = {
        name: nc.declare_dram_parameter(name, shape, dt, isOutput=False)
        for name, (shape, dt) in pshapes.items()
    }
    outT = nc.declare_dram_parameter("outT", [2 * D, ng2 * BT], F32, isOutput=True)

    with tile.TileContext(nc) as tc:
        with (
            tc.tile_pool(name="consts", bufs=1) as consts,
            tc.tile_pool(name="zv4", bufs=4) as zv4_pool,
            tc.tile_pool(name="hs", bufs=3) as hs_pool,
            tc.tile_pool(name="acat", bufs=3) as acat_pool,
            tc.tile_pool(name="ycat", bufs=3) as ycat_pool,
            tc.tile_pool(name="sq", bufs=3) as sq_pool,
            tc.tile_pool(name="h1s", bufs=nt) as h1s_pool,
            tc.tile_pool(name="gate", bufs=1) as gate_pool,
            tc.tile_pool(name="a1g", bufs=3) as a1g_pool,
            tc.tile_pool(name="outs", bufs=2) as outs_pool,
            tc.tile_pool(name="ps", bufs=3, space=bass.MemorySpace.PSUM) as ps_pool,
            tc.tile_pool(name="pv", bufs=3, space=bass.MemorySpace.PSUM) as pv_pool,
            tc.tile_pool(name="psn", bufs=1, space=bass.MemorySpace.PSUM) as psn_pool,
            tc.tile_pool(name="warm", bufs=1, space=bass.MemorySpace.PSUM) as warm_pool,
        ):
            # ---- HAM trigger: zero-weight matmuls need no DMA'd data, so
            # the PE clock gate opens while constants stream in ----
            wscr = consts.tile([H, BT], BF16, name="wscr")
            nc.vector.memset(wscr[:], 0.0)
            wps = warm_pool.tile([H, BT], F32, name="wps", tag="warm")

            def warm(n, cols=BT):
                for _ in range(n):
                    nc.tensor.matmul(wps[:, 0:cols], wscr[:, 0:H], wscr[:, 0:cols],
                                     start=True, stop=True)

            warm(12)

            # ---- load constants into SBUF ----
            ct = {}
            for name, (shape, dt) in pshapes.items():
                t = consts.tile(shape, dt, name=f"c_{name}")
                nc.sync.dma_start(t[:], params[name][:])
                ct[name] = t
            zero_b = consts.tile([nt, 1], F32, name="zero_b")
            nc.vector.memset(zero_b[:], 0.0)
            tanh_b = consts.tile([nt, 1], F32, name="tanh_b")
            nc.vector.memset(tanh_b[:], 0.5 * EPS)
            # tanh rows 0..nt-1 + constant-ones row nt (the +0.5 path);
            # memset the whole tile (single-partition writes at base 16 are
            # rejected by the BIR verifier), tanh later overwrites rows 0..15
            t17 = consts.tile([nt + 1, BT], BF16, name="t17")
            nc.vector.memset(t17[:], 1.0)

            taylor = [
                (ct["LT_t4"], ct["LT_b4"]),
                (ct["LT_t3"], ct["LT_b3"]),
                (ct["LT_t2"], ct["LT_b2k"]),
            ]

            h1s_tiles = []
            nsq_ps = psn_pool.tile([nt, BT], F32, name="nsq_ps", tag="nsq")

            # ================= phase A =================
            # tiles are emitted in pairs, stage-interleaved, so the PE always
            # has the partner tile's matmuls to run while DVE does the y-muls
            for tp in range(0, nt, 2):
                pair = [t for t in (tp, tp + 1) if t < nt]
                zv4s, hss, acats, ycats = {}, {}, {}, {}

                for t in pair:
                    zv4 = zv4_pool.tile([H, BT], BF16, name="zv4")
                    for r in range(4):
                        nc.sync.dma_start(zv4[32 * r:32 * (r + 1), :], zT[:, bass.ts(t, BT)])
                    zv4s[t] = zv4

                for t in pair:
                    hp = ps_pool.tile([H, BT], F32, name="hp", tag="ps")
                    nc.tensor.matmul(hp[:], ct["LT_h"][:], zv4s[t][0:D, :], start=True, stop=True)
                    hs = hs_pool.tile([H, BT], BF16, name="hs")
                    if sim_safe:
                        sg = hs_pool.tile([H, BT], F32, name="sg")
                        nc.scalar.activation(sg[:], hp[:], AF.Sigmoid, bias=ct["Bse1"][:])
                        hx = hs_pool.tile([H, BT], F32, name="hx")
                        nc.scalar.activation(hx[:], hp[:], AF.Identity, bias=ct["Bse1"][:])
                        nc.vector.tensor_tensor(hs[:], sg[:], hx[:], mybir.AluOpType.mult)
                    else:
                        nc.scalar.activation(hs[:], hp[:], AF.Silu, bias=ct["Bse1"][:])
                    hss[t] = hs

                for t in pair:
                    apt = ps_pool.tile([H, BT], F32, name="apt", tag="ps")
                    nc.tensor.matmul(apt[:], ct["LT_At"][:], hss[t][:], start=True, stop=True)
                    apb = ps_pool.tile([H, BT], F32, name="apb", tag="ps")
                    nc.tensor.matmul(apb[:], ct["LT_Ab"][:], hss[t][:], start=True, stop=True)
                    acat = acat_pool.tile([H, 2, BT], BF16, name="acat")
                    nc.scalar.activation(acat[:, 0, :], apt[:], AF.Identity, bias=ct["Bse2t"][:])
                    nc.scalar.activation(acat[:, 1, :], apb[:], AF.Identity, bias=ct["Bse2b"][:])
                    acats[t] = acat

                for t in pair:
                    # all-SBUF bf16 multiply: runs on GpSimd to keep DVE free
                    # for the PSUM-sourced taylor multiplies
                    ycat = ycat_pool.tile([H, 2, BT], BF16, name="ycat0")
                    nc.gpsimd.tensor_tensor(
                        ycat[:], acats[t][:],
                        zv4s[t][:, None, :].broadcast_to([H, 2, BT]),
                        mybir.AluOpType.mult,
                    )
                    ycats[t] = ycat

                for step_i, (lt_top, lt_bot) in enumerate(taylor):
                    pvs = {}
                    for t in pair:
                        pv = pv_pool.tile([H, BT], F32, name="pv", tag="pv")
                        nc.tensor.matmul(pv[:], lt_top[:], ycats[t][:, 0, :], start=True, stop=False)
                        nc.tensor.matmul(pv[:], lt_bot[:], ycats[t][:, 1, :], start=False, stop=False)
                        nc.tensor.matmul(pv[:], ct["LT_z"][:], zv4s[t][0:D, :], start=False, stop=True)
                        pvs[t] = pv
                    for t in pair:
                        ycat = ycat_pool.tile([H, 2, BT], BF16, name="ycat")
                        nc.vector.tensor_tensor(
                            ycat[:], acats[t][:],
                            pvs[t][:, None, :].broadcast_to([H, 2, BT]),
                            mybir.AluOpType.mult,
                        )
                        ycats[t] = ycat

                h1ps = {}
                for t in pair:
                    h1p = pv_pool.tile([H, BT], F32, name="h1p", tag="pv")
                    nc.tensor.matmul(h1p[:], ct["LT_t1"][:], ycats[t][:, 0, :], start=True, stop=False)
                    nc.tensor.matmul(h1p[:], ct["LT_b1k"][:], ycats[t][:, 1, :], start=False, stop=False)
                    nc.tensor.matmul(h1p[:], ct["LT_W1z"][:], zv4s[t][0:D, :], start=False, stop=True)
                    h1ps[t] = h1p

                for t in pair:
                    h1s = h1s_pool.tile([H, BT], BF16, name="h1s")
                    nc.scalar.activation(h1s[:], h1ps[t][:], AF.Identity, bias=ct["B1"][:])
                    sq = sq_pool.tile([H, BT], BF16, name="sq")
                    nc.scalar.activation(sq[:], h1ps[t][:], AF.Square, bias=ct["B1"][:])
                    h1s_tiles.append(h1s)
                    nc.tensor.matmul(
                        nsq_ps[:], ct["ONSQ"][:, bass.ts(t, nt)], sq[:],
                        start=(t == 0), stop=(t == nt - 1), skip_group_check=True,
                    )

            # ============== gate (batched sqrt + tanh) ==============
            # the two ACT table switches (~2.7us each) are the one PE bubble;
            # bridge it so the clock gate stays at K=8/8 for phase B
            warm(30)
            rt_all = gate_pool.tile([nt, BT], F32, name="rt_all")
            nc.scalar.activation(rt_all[:], nsq_ps[:], AF.Sqrt, bias=zero_b[:])
            # sigmoid(norm + eps) = 0.5 tanh(0.5 norm + eps/2) + 0.5
            nc.scalar.activation(t17[0:nt, :], rt_all[:], AF.Tanh, bias=tanh_b[:], scale=0.5)

            # ================= phase B =================
            outg = None
            for t in range(nt):
                g, r = divmod(t, 2)
                trp = ps_pool.tile([H, BT], F32, name="trp", tag="ps")
                nc.tensor.matmul(
                    trp[:], ct["E_sig"][:, bass.ts(t, H)], t17[:],
                    start=True, stop=True,
                )
                a1g = a1g_pool.tile([H, BT], BF16, name="a1g")
                nc.vector.tensor_tensor(
                    a1g[:], h1s_tiles[t][:], trp[:], mybir.AluOpType.mult
                )
                if r == 0:
                    outg = pv_pool.tile([H, BT], F32, name="outg", tag="pv")
                nc.tensor.matmul(
                    outg[32 * r:32 * (r + 1), :], ct["LT_W2"][:], a1g[:],
                    start=True, stop=True, skip_group_check=True,
                )
                if r == 1:
                    outs = outs_pool.tile([2 * D, BT], F32, name="outs")
                    nc.scalar.activation(outs[:], outg[0:2 * D, :], AF.Identity, bias=ct["B2_2"][:])
                    nc.sync.dma_start(outT[:, bass.ts(g, BT)], outs[:])

    if split_waits:
        _split_multi_waits(nc)
    return nc


def _host_params(G, W_se1, b_se1, W_se2, b_se2, W1, b1, W2, b2, nt):
    import ml_dtypes
    f = np.float32
    bf = ml_dtypes.bfloat16
    G = np.asarray(G, f)
    Gflat = np.transpose(G, (0, 2, 1)).reshape(NG * D, D)  # [(g,i), j] = G[g,j,i]
    W1G = Gflat @ np.asarray(W1, f).T                      # [(g,i), m]
    e_sig = np.zeros((nt + 1, nt * H), f)
    for t in range(nt):
        e_sig[t, t * H:(t + 1) * H] = 0.5
    e_sig[nt, :] = 0.5
    p = {
        "LT_h": np.asarray(W_se1, f).T,
        "LT_At": np.repeat(np.asarray(W_se2, f).T[:, 0:4], 32, axis=1),
        "LT_Ab": np.repeat(np.asarray(W_se2, f).T[:, 4:8], 32, axis=1),
        "Bse1": np.asarray(b_se1, f).reshape(H, 1),
        "Bse2t": np.repeat(np.asarray(b_se2, f)[0:4], 32).reshape(H, 1),
        "Bse2b": np.repeat(np.asarray(b_se2, f)[4:8], 32).reshape(H, 1),
        "LT_z": np.tile(np.eye(D, dtype=f), (1, 4)),
        "LT_W1z": np.asarray(W1, f).T,
        "B1": np.asarray(b1, f).reshape(H, 1),
        "LT_W2": np.asarray(W2, f).T,
        "B2_2": np.tile(np.asarray(b2, f), 2).reshape(2 * D, 1),
        "LT_t1": np.ascontiguousarray(W1G[:H]),
        "LT_b1k": np.ascontiguousarray(W1G[H:]),
        "E_sig": e_sig,
        "ONSQ": np.tile(np.eye(nt, dtype=f).reshape(1, nt * nt), (H, 1)),
    }
    for k, tname, bname in ((4, "LT_t4", "LT_b4"), (3, "LT_t3", "LT_b3"), (2, "LT_t2", "LT_b2k")):
        scaled = np.tile(Gflat * f(1.0 / k), (1, 4))
        p[tname] = np.ascontiguousarray(scaled[:H])
        p[bname] = np.ascontiguousarray(scaled[H:])
    for name, (shape, dt) in _param_shapes(nt).items():
        arr = np.ascontiguousarray(p[name])
        p[name] = arr.astype(bf) if dt == BF16 else arr.astype(f)
    return p


def _run(z, G, W_se1, b_se1, W_se2, b_se2, W1, b1, W2, b2, trace=False, **trace_kw):
    import ml_dtypes
    z = np.asarray(z, np.float32)
    nt = BC // BT
    params = _host_params(G, W_se1, b_se1, W_se2, b_se2, W1, b1, W2, b2, nt)

    # shard: per-core feature-major bf16 slices
    zT = np.ascontiguousarray(
        z.reshape(NCORES, BC, D).transpose(0, 2, 1).astype(ml_dtypes.bfloat16)
    )

    nc = _build_program(BC)
    in_maps = [{"zT": zT[c], **params} for c in range(NCORES)]
    res = run_bass_kernel_spmd(nc, in_maps, list(range(NCORES)), trace=trace, **trace_kw)

    # outT[32r+d, g*BT+b] = out[(2g+r)*BT + b, d] per core
    outT = np.stack([res.results[c]["outT"] for c in range(NCORES)])
    out = (
        outT.reshape(NCORES, 2, D, nt // 2, BT)
        .transpose(0, 3, 1, 4, 2)
        .reshape(B, D)
    )
    return np.ascontiguousarray(out.astype(np.float32)), res


def kernel(z, G, W_se1, b_se1, W_se2, b_se2, W1, b1, W2, b2):
    out, _ = _run(z, G, W_se1, b_se1, W_se2, b_se2, W1, b1, W2, b2, trace=False)
    return out


if __name__ == "__main__":
    rng = np.random.default_rng(0)
    inputs = {
        "z": rng.standard_normal((B, D), dtype=np.float32),
        "G": (rng.standard_normal((NG, D, D)) * 0.1).astype(np.float32),
        "W_se1": (rng.standard_normal((H, D)) / np.sqrt(D)).astype(np.float32),
        "b_se1": np.zeros(H, np.float32),
        "W_se2": (rng.standard_normal((NG, H)) / np.sqrt(H)).astype(np.float32),
        "b_se2": np.zeros(NG, np.float32),
        "W1": (rng.standard_normal((H, D)) * 0.01).astype(np.float32),
        "b1": np.zeros(H, np.float32),
        "W2": (rng.standard_normal((D, H)) * 0.01).astype(np.float32),
        "b2": np.zeros(D, np.float32),
    }
    out = kernel(**inputs)
    print("kernel output", out.shape, out.dtype, float(np.abs(out).max()))
